# revision 18
# baseline (speedup 1.0000x reference)
"""CARFAC cell kernel for 8 TRN2 NeuronCores.

Math: y[b,c,n] is the linear recurrence a[n+1] = f[n+1]*a[n] + g[n+1]
(computed exactly with the DVE's tensor_tensor_scan instruction — the
reference's cumsum-of-logs + triangular-matmul expansion is just a
parallel-friendly expression of the same recurrence), followed by
`steps` rounds of a symmetric-padded 3-tap FIR across channels.

Key identity for the smoothing stage: half-sample symmetric padding
commutes with a symmetric FIR, so applying the 3-tap kernel `steps`
times equals ONE conv with the `steps`-fold self-convolution of the
kernel (17 taps for steps=8) on the reflect-extended signal. That
collapses to a single [C x C] matrix W (banded + boundary-folded),
i.e. one TensorEngine matmul.

Sharding: 8 cores = 2 batches x 4 channel-quarters. Each core loads its
owned ~18 channels plus an 8-channel halo (<=34 rows of f/g), scans the
recurrence for all loaded rows, and applies its [34 x 18] slice of W
(halo selection + reflection encoded host-side in the weights). No
cross-core communication of any kind.

Performance notes (from neuron-profile traces):
- A dynamic DMA's descriptors are processed by ONE SDMA engine
  (~27 GB/s = one SBUF port); the sync, scalar and gpsimd DGE paths
  are independent, so transfers are split across all three.
- The input is loaded in two waves ordered [f0|g0|a0|W | f1|g1] so the
  first scan half starts while the second half is still in flight.
- Raw Bass (no Tile, no Block): Tile's tail drain exceeds the HW's
  per-instruction sync-wait cap, and Block's exit all-engine barrier
  costs ~4 us of pure epilogue.
"""

import numpy as np

B, C, N = 2, 71, 1024
NCORES = 8
QPB = 4  # channel-quarters per batch element
HALO = 8  # channel reach of the smoothing: steps * (ksz-1)//2
ROWS = 34  # rows loaded per core: own(<=18) + up to 2*HALO, padded
OWN = 18  # owned output channels per core (last quarter uses 17)

_OWN_LO = [0, 18, 36, 54]
_OWN_SZ = [18, 18, 18, 17]

HALF = 512  # one PSUM bank of fp32 per matmul
# Packed input, wave A then wave B: [f0 | g0 | a0 | w] + [f1 | g1]
_F0, _G0, _A0, _W0 = 0, HALF, 2 * HALF, 2 * HALF + 1
_WAVE_B = _W0 + OWN  # 1043
_F1, _G1 = _WAVE_B, _WAVE_B + HALF
PACK = _WAVE_B + 2 * HALF  # 2067

FP32R = True  # single-pass PE matmul (4x faster); flip off if accuracy drops

_PROGRAM = None


def _build_program():
    import concourse.bass as bass
    import concourse.mybir as mybir

    f32 = mybir.dt.float32
    mm_dt = mybir.dt.float32r if FP32R else f32
    mult, add = mybir.AluOpType.mult, mybir.AluOpType.add
    nc = bass.Bass(enable_partition_id=False)
    in_loc = nc.declare_dram_parameter("in_loc", [ROWS, PACK], f32, isOutput=False)
    out_loc = nc.declare_dram_parameter("out_loc", [OWN, N], f32, isOutput=True)

    Q, CL = 4, N // 4  # scan/matmul pipeline chunks

    with (
        nc.sbuf_tensor([ROWS, PACK], f32) as it,
        nc.sbuf_tensor([ROWS, N], mm_dt) as yt,
        nc.sbuf_tensor([ROWS, OWN], mm_dt) as wr,
        nc.sbuf_tensor([OWN, N], f32) as ot,
        nc.psum_tensor([OWN, CL], f32) as ps0,
        nc.psum_tensor([OWN, CL], f32) as ps1,
        nc.psum_tensor([OWN, CL], f32) as ps2,
        nc.psum_tensor([OWN, CL], f32) as ps3,
        nc.semaphore("a_hw") as a_hw,  # wave A, HWDGE (sync+scalar)
        nc.semaphore("a_sw") as a_sw,  # wave A, SWDGE (gpsimd)
        nc.semaphore("b_hw") as b_hw,
        nc.semaphore("b_sw") as b_sw,
        nc.semaphore("o_hw") as o_hw,  # output stores
        nc.semaphore("o_sw") as o_sw,
        nc.semaphore("v_sem") as v_sem,  # DVE scans
        nc.semaphore("p_sem") as p_sem,  # PE matmuls
        nc.semaphore("c_sem") as c_sem,  # ACT PSUM->SBUF copies
        nc.semaphore("w_sem") as w_sem,  # W staged as FP32r
    ):
        a0t = it[:, _A0 : _A0 + 1]
        wt = it[:, _W0 : _W0 + OWN]
        ps = [ps0, ps1, ps2, ps3]

        def fcol(t):  # f column in the packed layout for global time t
            return _F0 + t if t < HALF else _F1 + (t - HALF)

        def gcol(t):
            return _G0 + t if t < HALF else _G1 + (t - HALF)

        # Input load: two waves, each split across the three DGE paths.
        SPLITS = [("sync", 0, 12), ("scalar", 12, 24), ("gpsimd", 24, 34)]
        for (c0, c1, hw_sem, sw_sem) in (
            (0, _WAVE_B, a_hw, a_sw),
            (_WAVE_B, PACK, b_hw, b_sw),
        ):
            for eng, r0, r1 in SPLITS:
                getattr(nc, eng).dma_start(
                    out=it[r0:r1, c0:c1], in_=in_loc[r0:r1, c0:c1]
                ).then_inc(sw_sem if eng == "gpsimd" else hw_sem, 16)

        # ACT: stage W as FP32r (the PE mode needs FP32r producers). Doing
        # this on ACT keeps it off the DVE critical path, and the first
        # ACTIVATE also absorbs the one-time ~1.3us ACT table load before
        # the PSUM-evacuation copies need it.
        nc.scalar.wait_ge(a_hw, 32)
        nc.scalar.wait_ge(a_sw, 16)
        nc.scalar.copy(wr[:, :], wt).then_inc(w_sem, 1)

        # DVE: the recurrence scan in Q chunks, chained via
        # initial=prev_out[:, -1:].
        nc.vector.wait_ge(a_hw, 32)
        nc.vector.wait_ge(a_sw, 16)
        for q in range(Q):
            t0, t1 = q * CL, (q + 1) * CL
            if t0 == HALF:  # chunks beyond HALF live in wave B
                nc.vector.wait_ge(b_hw, 32)
                nc.vector.wait_ge(b_sw, 16)
            init = a0t if q == 0 else yt[:, t0 - 1 : t0]
            if q:
                nc.vector.wait_ge(v_sem, q)  # carry readable (race det.)
            nc.vector.tensor_tensor_scan(
                yt[:, t0:t1],
                it[:, fcol(t0) : fcol(t0) + CL],
                it[:, gcol(t0) : gcol(t0) + CL],
                init,
                op0=mult,
                op1=add,
            ).then_inc(v_sem, 1)

        # PE: one smoothing matmul per chunk.
        nc.tensor.wait_ge(w_sem, 1)
        for q in range(Q):
            nc.tensor.wait_ge(v_sem, q + 1)
            nc.tensor.matmul(
                ps[q][:, :],
                wr[:, :],
                yt[:, q * CL : (q + 1) * CL],
                start=True,
                stop=True,
            ).then_inc(p_sem, 1)

        # ACT: evacuate PSUM per chunk (runs parallel to later scans).
        for q in range(Q):
            nc.scalar.wait_ge(p_sem, q + 1)
            nc.scalar.copy(ot[:, q * CL : (q + 1) * CL], ps[q][:, :]).then_inc(
                c_sem, 1
            )

        # Stores per quarter on the sync + gpsimd rings (scalar is busy
        # copying); only the last quarter's store is left on the tail.
        for q in range(Q):
            cols = slice(q * CL, (q + 1) * CL)
            for eng, r0, r1 in (("sync", 0, 9), ("gpsimd", 9, 18)):
                e = getattr(nc, eng)
                e.wait_ge(c_sem, q + 1)
                e.dma_start(out=out_loc[r0:r1, cols], in_=ot[r0:r1, cols]).then_inc(
                    o_sw if eng == "gpsimd" else o_hw, 16
                )
        nc.sync.wait_ge(o_hw, 16 * Q)
        nc.gpsimd.wait_ge(o_sw, 16 * Q)

    return nc


def _strip_framework_preamble(nc):
    """Drop the framework preamble's const memsets, engine drains and the
    all-engine EVSEM barrier (~4 us on the critical path). Everything in
    this kernel is gated on data semaphores, so engines starting skewed is
    fine. Serialization-level: patches this instance's to_json_bytes."""
    import orjson

    m = nc.to_json()
    for fn in m["functions"]:
        for blk in fn["blocks"]:
            blk["instructions"] = [
                i
                for i in blk["instructions"]
                if not (
                    i.get("opcode") in ("Memset", "Drain")
                    or str(i.get("name", "")).startswith("barrier_")
                )
            ]
    payload = orjson.dumps(m)
    nc.to_json_bytes = lambda: payload
    return nc


def _conv_matrix(kernel: np.ndarray, steps: int) -> np.ndarray:
    """[C, C] matrix equivalent to `steps` rounds of symmetric-pad conv."""
    eff = np.array([1.0], np.float64)
    for _ in range(steps):
        eff = np.convolve(eff, kernel.astype(np.float64))
    h = (len(eff) - 1) // 2
    assert h <= HALO, f"kernel reach {h} exceeds layout halo {HALO}"
    W = np.zeros((C, C), np.float64)
    for c in range(C):
        for d in range(-h, h + 1):
            idx = c + d
            if idx < 0:
                idx = -1 - idx
            if idx >= C:
                idx = 2 * C - 1 - idx
            W[idx, c] += eff[d + h]
    return W.astype(np.float32)


def _pack_core(core: int, a_0, f, g, W):
    """Build one core's packed [ROWS, PACK] input; returns (in_loc, b, lo, sz)."""
    b, q = divmod(core, QPB)
    lo, sz = _OWN_LO[q], _OWN_SZ[q]
    r0 = max(0, lo - HALO)
    r1 = min(C, lo + sz + HALO)
    nr = r1 - r0

    in_loc = np.zeros((ROWS, PACK), np.float32)
    in_loc[:, _F0 : _F0 + HALF] = 0.5  # benign f for padded rows
    in_loc[:, _F1 : _F1 + HALF] = 0.5
    in_loc[:nr, _F0 : _F0 + HALF] = f[b, r0:r1, :HALF]
    in_loc[:nr, _F1 : _F1 + HALF] = f[b, r0:r1, HALF:]
    in_loc[:nr, _G0 : _G0 + HALF] = g[b, r0:r1, :HALF]
    in_loc[:nr, _G1 : _G1 + HALF] = g[b, r0:r1, HALF:]
    in_loc[:nr, _A0] = a_0[b, r0:r1]
    in_loc[:nr, _W0 : _W0 + sz] = W[r0:r1, lo : lo + sz]
    return in_loc, b, lo, sz


LAST_RESULT = None  # BassKernelResults of the most recent run (for test.py)
TRACE = False  # set True (e.g. by test.py) to capture an NTFF profile


def kernel(a_0, f, g, kernel, steps):
    global _PROGRAM, LAST_RESULT
    from concourse.bass_utils import run_bass_kernel_spmd

    a_0 = np.asarray(a_0, np.float32)
    f = np.asarray(f, np.float32)
    g = np.asarray(g, np.float32)
    W = _conv_matrix(np.asarray(kernel), int(steps))

    in_maps = []
    meta = []
    for core in range(NCORES):
        in_loc, b, lo, sz = _pack_core(core, a_0, f, g, W)
        in_maps.append({"in_loc": in_loc})
        meta.append((b, lo, sz))

    if _PROGRAM is None:
        _PROGRAM = _strip_framework_preamble(_build_program())

    res = run_bass_kernel_spmd(
        _PROGRAM, in_maps, core_ids=list(range(NCORES)), trace=TRACE
    )
    LAST_RESULT = res

    out = np.empty((B, C, N), np.float32)
    for core, (b, lo, sz) in enumerate(meta):
        out[b, lo : lo + sz] = res.results[core]["out_loc"][:sz]
    return out
